# revision 24
# baseline (speedup 1.0000x reference)
"""Trainium2 Bass kernel for nn_Caps_BN (BatchNorm2d + grouped 1x1 conv).

Reference computation (per full input x of shape (64, 512, 32, 32)):
    mean/var per channel over (N, H, W)  [training-mode biased BN, affine=False]
    xn = (x - mean) * rsqrt(var + eps)
    out[n, (c,o), hw] = sum_i W[c, o, i] * xn[n, (c,i), hw] + bias[(c,o)]

Strategy:
  * Data-parallel over the batch dim: 8 cores x 8 batches each.
  * BN is folded into the conv:  out = W' @ x + bias', where
        W'[c,o,i]  = W[c,o,i] * rsqrt(var[c,i] + eps)
        bias'[c,o] = bias[c,o] - sum_i W'[c,o,i] * mean[c,i]
    so the kernel never materializes xn — a single matmul pass over raw x.
  * Per-channel (sum, sumsq) are computed locally with bn_stats/bn_aggr
    (one DVE pass over resident data), then a 4 KB AllReduce combines them
    across the 8 cores.
  * Channels are processed in 4 groups of 128 (= 4 capsules of D=32); each
    group's weights form a block-diagonal 128x128 lhsT so the TensorEngine
    contracts over the full 128-partition dim.
"""

import sys

if "/opt/trn_rl_repo" not in sys.path:
    sys.path.insert(0, "/opt/trn_rl_repo")

import numpy as np

import concourse.bass as bass
import concourse.bacc as bacc
import concourse.mybir as mybir
import concourse.tile as tile
from concourse.bass_utils import run_bass_kernel_spmd

N_CORES = 8
N_FULL = 64
C, D = 16, 32
CD = C * D  # 512 channels
H = W = 32
HW = H * W  # 1024
NL = N_FULL // N_CORES  # batches per core
G = CD // 128  # channel groups of 128 (= 4 capsules each)
CPG = 128 // D  # capsules per group (4)
FC = 512  # matmul moving-operand chunk (fp32 max / one PSUM bank)
EPS = 1e-5

F32 = mybir.dt.float32
ALU = mybir.AluOpType
ACTF = mybir.ActivationFunctionType

# Matmul compute dtype for the main conv loop. float32r streams fp32
# operands through the PE in a single pass (4x the fp32 rate at FD>=256)
# with reduced-precision multiplies; float32 is the exact 2-pass mode.
MM_DTYPE = mybir.dt.float32r


def build_nc(nl: int = NL, n_cores: int = N_CORES, copy_split: int = 2):
    """Build the SPMD Bass program (identical on every core).

    copy_split: every copy_split-th PSUM->SBUF bias-add copy goes to the
    Scalar engine (ACT Identity) instead of DVE; 0 = all on DVE.
    """
    f = nl * HW  # free-dim elements per channel group
    ntot = float(n_cores * nl * HW)  # BN population per channel
    n_chunks = f // FC

    nc = bacc.Bacc(
        "TRN2", target_bir_lowering=False, debug=False, num_devices=n_cores
    )
    # x and the folded weight are typed as the matmul compute dtype
    # (float32r = same 4-byte fp32 bits, single-pass PE mode); all
    # non-matmul consumers view them through .bitcast(F32).
    x_d = nc.dram_tensor("x_shard", [nl, CD, HW], MM_DTYPE, kind="ExternalInput")
    # lhsT_bd is the host-prepared block-diagonal transposed weight:
    # lhsT_bd[g, cl*D+i, cl*D+o] = weight[g*CPG+cl, o, i], zero off-block.
    w_d = nc.dram_tensor("lhsT_bd", [G, 128, 128], MM_DTYPE, kind="ExternalInput")
    b_d = nc.dram_tensor("bias", [CD], F32, kind="ExternalInput")
    o_d = nc.dram_tensor("out", [nl, CD, HW], F32, kind="ExternalOutput")

    with tile.TileContext(nc) as tc:
        with (
            tc.tile_pool(name="xp", bufs=1) as xp,
            tc.tile_pool(name="wp", bufs=1) as wp,
            tc.tile_pool(name="st", bufs=1) as st,
            tc.tile_pool(name="stage", bufs=2) as sp,
            tc.tile_pool(name="ps", bufs=6, space="PSUM") as pp,
            tc.tile_pool(name="psb", bufs=2, space="PSUM") as ppb,
            tc.tile_pool(name="dram", bufs=1, space="DRAM") as dp,
        ):
            # x viewed as (group, channel-in-group, batch, hw)
            xr = x_d.rearrange("n (g p) f -> g p n f", p=128)
            orr = o_d.rearrange("n (g p) f -> g p n f", p=128)
            pairs = [(0, 1), (2, 3)]  # channel-group pairs, one AllReduce each

            # ---- weights: one DMA per group for the block-diag lhsT ----
            lhsT = []
            for j in range(G):
                lt = wp.tile([128, 128], MM_DTYPE, tag=f"lhsT{j}", name=f"lhsT{j}")
                nc.sync.dma_start(out=lt[:, :], in_=w_d[j])
                lhsT.append(lt)

            bias_sb = []
            br = b_d.rearrange("(g p one) -> g p one", p=128, one=1)
            for j in range(G):
                bt = st.tile([128, 1], F32, tag=f"bias{j}", name=f"bias{j}")
                nc.sync.dma_start(out=bt[:, :], in_=br[j])
                bias_sb.append(bt)

            epst = st.tile([128, 1], F32, tag="epst", name="epst")
            nc.vector.memset(epst[:, :], EPS)
            zt = st.tile([128, 1], F32, tag="zt", name="zt")
            nc.vector.memset(zt[:, :], 0.0)

            # ---- load x (both HWDGE rings), local (sum, sumsq) ---------
            # spack[P] layout: [sum_g0, sum_g1, sumsq_g0, sumsq_g1]
            xt = []
            spack = []
            scr = st.tile([128, f], F32, tag="scr", name="scr")
            for P, pair in enumerate(pairs):
                sp_t = st.tile([128, 4], F32, tag=f"spack{P}", name=f"spack{P}")
                spack.append(sp_t)
            for j in range(G):
                t = xp.tile([128, f], MM_DTYPE, tag=f"x{j}", name=f"x{j}")
                eng = nc.sync if j % 2 == 0 else nc.scalar
                eng.dma_start(out=t.rearrange("p (n f) -> p n f", n=nl), in_=xr[j])
                xt.append(t)
                P, jl = divmod(j, 2)
                # per-channel sum on DVE (single full-width reduce)
                nc.vector.tensor_reduce(
                    out=spack[P][:, jl : jl + 1],
                    in_=t[:, :].bitcast(F32),
                    axis=mybir.AxisListType.X,
                    op=ALU.add,
                )
                # per-channel sum of squares on ACT (Square + accumulate)
                nc.scalar.activation(
                    scr[:, :],
                    t[:, :].bitcast(F32),
                    ACTF.Square,
                    bias=zt[:, :],
                    accum_out=spack[P][:, 2 + jl : 3 + jl],
                )

            # ---- per-pair: AllReduce, fold stats, matmul, store --------
            for P, pair in enumerate(pairs):
                cc_in = dp.tile([128, 4], F32, tag=f"ccin{P}", name=f"ccin{P}")
                cc_out = dp.tile([128, 4], F32, tag=f"ccout{P}", name=f"ccout{P}")
                nc.gpsimd.dma_start(out=cc_in[:, :], in_=spack[P][:, :])
                nc.gpsimd.collective_compute(
                    "AllReduce",
                    ALU.add,
                    replica_groups=[list(range(n_cores))],
                    ins=[cc_in.opt()],
                    outs=[cc_out.opt()],
                )
                sg = st.tile([128, 4], F32, tag=f"sg{P}", name=f"sg{P}")
                nc.gpsimd.dma_start(out=sg[:, :], in_=cc_out[:, :])

                for jl, j in enumerate(pair):
                    mean = st.tile([128, 1], F32, tag=f"gmean{j}", name=f"gmean{j}")
                    nc.vector.tensor_scalar_mul(
                        mean[:, :], sg[:, jl : jl + 1], 1.0 / ntot
                    )
                    ex2 = st.tile([128, 1], F32, tag=f"gex2{j}", name=f"gex2{j}")
                    nc.vector.tensor_scalar_mul(
                        ex2[:, :], sg[:, 2 + jl : 3 + jl], 1.0 / ntot
                    )
                    msq = st.tile([128, 1], F32, tag=f"gmsq{j}", name=f"gmsq{j}")
                    nc.vector.tensor_tensor(
                        msq[:, :], mean[:, :], mean[:, :], ALU.mult
                    )
                    var = st.tile([128, 1], F32, tag=f"gvar{j}", name=f"gvar{j}")
                    nc.vector.tensor_tensor(
                        var[:, :], ex2[:, :], msq[:, :], ALU.subtract
                    )
                    sd = st.tile([128, 1], F32, tag=f"gsd{j}", name=f"gsd{j}")
                    nc.scalar.activation(
                        sd[:, :], var[:, :], ACTF.Sqrt, bias=epst[:, :]
                    )
                    rs = st.tile([128, 1], F32, tag=f"grs{j}", name=f"grs{j}")
                    nc.vector.reciprocal(rs[:, :], sd[:, :])
                    # scale lhsT rows by rsqrt(var+eps) of the input channel
                    nc.vector.tensor_scalar_mul(
                        lhsT[j][:, :], lhsT[j][:, :].bitcast(F32), rs[:, :]
                    )
                    nmean = st.tile([128, 1], F32, tag=f"gnm{j}", name=f"gnm{j}")
                    nc.vector.tensor_scalar_mul(nmean[:, :], mean[:, :], -1.0)
                    # bias' = bias - W' @ mean  (block-diag matmul, K=128)
                    pb = ppb.tile([128, 1], F32, tag="pbias", name=f"pbias{j}")
                    nc.tensor.matmul(
                        pb[:, :],
                        lhsT[j][:, :].bitcast(F32),
                        nmean[:, :],
                        start=True,
                        stop=True,
                    )
                    bp = st.tile([128, 1], F32, tag=f"gbp{j}", name=f"gbp{j}")
                    nc.vector.tensor_tensor(
                        bp[:, :], pb[:, :], bias_sb[j][:, :], ALU.add
                    )

                    # ---- grouped conv for this channel group ----------
                    n_blk = 2 if nl >= 2 else 1
                    cpb = n_chunks // n_blk  # chunks per stage block
                    for b in range(n_blk):
                        stg = sp.tile(
                            [128, f // n_blk], F32, tag="stage", name=f"stage{j}_{b}"
                        )
                        for c in range(cpb):
                            ch = b * cpb + c
                            ps = pp.tile([128, FC], F32, tag="ps", name=f"ps{j}_{ch}")
                            nc.tensor.matmul(
                                ps[:, :],
                                lhsT[j][:, :],
                                xt[j][:, ch * FC : (ch + 1) * FC],
                                start=True,
                                stop=True,
                            )
                            if copy_split and (c % copy_split == copy_split - 1):
                                nc.scalar.activation(
                                    stg[:, c * FC : (c + 1) * FC],
                                    ps[:, :],
                                    ACTF.Identity,
                                    bias=bp[:, :],
                                )
                            else:
                                nc.vector.tensor_scalar_add(
                                    stg[:, c * FC : (c + 1) * FC], ps[:, :], bp[:, :]
                                )
                        orj = orr[j]  # (128, nl, HW)
                        sgr = stg.rearrange("p (n f) -> p n f", n=nl // n_blk)
                        nb = nl // n_blk
                        eng = nc.sync if (j + b) % 2 == 0 else nc.scalar
                        eng.dma_start(
                            out=orj[:, b * nb : (b + 1) * nb, :], in_=sgr
                        )

    nc.compile()
    return nc


_NC_CACHE: dict = {}


def _get_nc(nl: int, n_cores: int):
    key = (nl, n_cores)
    if key not in _NC_CACHE:
        _NC_CACHE[key] = build_nc(nl=nl, n_cores=n_cores)
    return _NC_CACHE[key]


def make_lhsT_bd(weight: np.ndarray) -> np.ndarray:
    lb = np.zeros((G, 128, 128), dtype=np.float32)
    for c in range(C):
        g, cl = divmod(c, CPG)
        s = cl * D
        lb[g, s : s + D, s : s + D] = weight[c].T  # (i, o)
    return lb


def make_in_maps(x, weight, bias):
    lhsT_bd = make_lhsT_bd(weight)
    return [
        {
            "x_shard": np.ascontiguousarray(
                x[i * NL : (i + 1) * NL].reshape(NL, CD, HW)
            ),
            "lhsT_bd": lhsT_bd,
            "bias": np.ascontiguousarray(bias),
        }
        for i in range(N_CORES)
    ]


def kernel(x: np.ndarray, weight: np.ndarray, bias: np.ndarray) -> np.ndarray:
    assert x.shape == (N_FULL, CD, H, W) and x.dtype == np.float32
    nc = _get_nc(NL, N_CORES)
    in_maps = make_in_maps(x, weight, bias)
    res = run_bass_kernel_spmd(nc, in_maps, core_ids=list(range(N_CORES)))
    out = np.concatenate(
        [res.results[i]["out"].reshape(NL, CD, H, W) for i in range(N_CORES)], axis=0
    )
    return out.astype(np.float32, copy=False)


# revision 30
# speedup vs baseline: 1.1607x; 1.1607x over previous
"""Trainium2 Bass kernel for nn_Caps_BN (BatchNorm2d + grouped 1x1 conv).

Reference computation (full input x of shape (64, 512, 32, 32)):
    mean/var per channel over (N, H, W)  [training-mode biased BN, affine=False]
    xn = (x - mean) * rsqrt(var + eps)
    out[n, (c,o), hw] = sum_i W[c, o, i] * xn[n, (c,i), hw] + bias[(c,o)]

Strategy — channel sharding, zero collectives:
  * Each of the 8 cores owns 2 capsules (64 channels) across the FULL batch,
    so BN statistics are entirely core-local: no AllReduce, which avoids the
    ~60-100us collective rendezvous floor measured on this part.
  * On-chip layout folds the batch into the partition dim: partition
    p = a*64 + ch with a = batch-half (n>=32), ch = local channel. All DMAs
    span 128 partitions (full bandwidth).
  * BN is folded into the conv:  out = W' @ x + bias', with
        W'[c,o,i]  = W[c,o,i] * rsqrt(var[c,i] + eps)
        bias'[c,o] = bias[c,o] - sum_i W'[c,o,i] * mean[c,i]
    so the kernel never materializes xn — one matmul pass over raw x.
  * The per-capsule weights form a host-prepared block-diagonal 128x128
    lhsT = diag(W_c0, W_c1, W_c0, W_c1) (duplicated for the two batch
    halves); the TensorEngine contracts the full 128-partition dim.
  * The two batch-half partial (sum, sumsq) are combined by a tiny matmul
    against a host-provided 0/1-pattern fold matrix (scaled 1/N) that also
    performs the /N, yielding per-channel (mean, E[x^2]) directly.
  * Matmuls run in float32r (single-pass fp32 PE mode).
"""

import sys

if "/opt/trn_rl_repo" not in sys.path:
    sys.path.insert(0, "/opt/trn_rl_repo")

import numpy as np

import concourse.bass as bass
import concourse.bacc as bacc
import concourse.mybir as mybir
import concourse.tile as tile
from concourse.bass_utils import run_bass_kernel_spmd

N_CORES = 8
N_FULL = 64
C, D = 16, 32
CD = C * D  # 512 channels
H = W = 32
HW = H * W  # 1024
CPC = C // N_CORES  # capsules per core (2)
CHL = CPC * D  # local channels per core (64)
FC = 512  # matmul moving-operand chunk (one PSUM bank of fp32)
EPS = 1e-5

F32 = mybir.dt.float32
ALU = mybir.AluOpType
ACTF = mybir.ActivationFunctionType

# float32r = same 4-byte fp32 bits, single-pass PE mode (4x fp32 rate at
# FD>=256) with reduced-precision multiplies; float32 = exact 2-pass mode.
MM_DTYPE = mybir.dt.float32r


def build_nc(n_full: int = N_FULL, n_cores: int = N_CORES, copy_split: int = 2):
    """Build the SPMD Bass program (identical on every core; per-core data
    differs: each core receives its own channel slice / weights)."""
    A = 2  # batch halves folded into the partition dim
    M = n_full // A  # batch entries per half
    f = M * HW  # free-dim elements per partition
    ntot = float(n_full * HW)  # BN population per channel
    n_pieces = 4 if M % 4 == 0 else 1  # input DMA / stats granularity
    m_pp = M // n_pieces
    n_chunks = f // FC
    n_blk = max(1, min(8, f // 4096))  # output stage blocks
    cpb = n_chunks // n_blk

    nc = bacc.Bacc(
        "TRN2", target_bir_lowering=False, debug=False, num_devices=n_cores
    )
    # Per-core shard: all batches, CHL local channels.
    x_d = nc.dram_tensor("x_shard", [n_full, CHL, HW], MM_DTYPE, kind="ExternalInput")
    # Host-prepared block-diagonal transposed weight (see make_core_inputs).
    w_d = nc.dram_tensor("lhsT_bd", [128, 128], MM_DTYPE, kind="ExternalInput")
    # Per-partition bias, duplicated across the two batch halves.
    b_d = nc.dram_tensor("bias_dup", [128], F32, kind="ExternalInput")
    # Fold matrix: fm[k, m] = 1/ntot iff k == m (mod 64); combines the two
    # batch-half partial sums and divides by N in one tiny matmul.
    fm_d = nc.dram_tensor("foldmat", [128, 128], F32, kind="ExternalInput")
    o_d = nc.dram_tensor("out", [n_full, CHL, HW], F32, kind="ExternalOutput")

    with tile.TileContext(nc) as tc:
        with (
            tc.tile_pool(name="xp", bufs=1) as xp,
            tc.tile_pool(name="wp", bufs=1) as wp,
            tc.tile_pool(name="st", bufs=1) as st,
            tc.tile_pool(name="stage", bufs=2) as sp,
            tc.tile_pool(name="ps", bufs=6, space="PSUM") as pp,
            tc.tile_pool(name="psb", bufs=1, space="PSUM") as ppb,
        ):
            # (a m) c f -> a c m f : partition = (half, channel); the two
            # halves are moved by separate DMAs on the two HWDGE rings
            # (disjoint SBUF port sets -> they drain concurrently).
            xr = x_d.rearrange("(a m) c f -> a c m f", a=A)
            orr = o_d.rearrange("(a m) c f -> a c m f", a=A)

            # ---- constants ------------------------------------------
            lt = wp.tile([128, 128], MM_DTYPE, tag="lhsT", name="lhsT")
            nc.sync.dma_start(out=lt[:, :], in_=w_d[:, :])
            fm = wp.tile([128, 128], F32, tag="foldmat", name="foldmat")
            nc.sync.dma_start(out=fm[:, :], in_=fm_d[:, :])
            bt = st.tile([128, 1], F32, tag="bias", name="bias")
            nc.sync.dma_start(
                out=bt[:, :], in_=b_d.rearrange("(p one) -> p one", one=1)
            )
            epst = st.tile([128, 1], F32, tag="epst", name="epst")
            nc.vector.memset(epst[:, :], EPS)
            zt = st.tile([128, 1], F32, tag="zt", name="zt")
            nc.vector.memset(zt[:, :], 0.0)

            # ---- load x in pieces; per-piece (sum, sumsq) -----------
            xt = xp.tile([128, f], MM_DTYPE, tag="x", name="xt")
            scr = st.tile([128, m_pp * HW], F32, tag="scr", name="scr")
            sumc = st.tile([128, n_pieces], F32, tag="sumc", name="sumc")
            sqc = st.tile([128, n_pieces], F32, tag="sqc", name="sqc")
            for q in range(n_pieces):
                lo, hi = q * m_pp * HW, (q + 1) * m_pp * HW
                for a, eng in ((0, nc.sync), (1, nc.scalar)):
                    eng.dma_start(
                        out=xt[a * 64 : (a + 1) * 64, lo:hi].rearrange(
                            "p (m f) -> p m f", f=HW
                        ),
                        in_=xr[a][:, q * m_pp : (q + 1) * m_pp, :],
                    )
                nc.vector.tensor_reduce(
                    out=sumc[:, q : q + 1],
                    in_=xt[:, lo:hi].bitcast(F32),
                    axis=mybir.AxisListType.X,
                    op=ALU.add,
                )
                nc.scalar.activation(
                    scr[:, :],
                    xt[:, lo:hi].bitcast(F32),
                    ACTF.Square,
                    bias=zt[:, :],
                    accum_out=sqc[:, q : q + 1],
                )

            # ---- combine partials -> mean / E[x^2] ------------------
            spack = st.tile([128, 2], F32, tag="spack", name="spack")
            nc.vector.tensor_reduce(
                out=spack[:, 0:1], in_=sumc[:, :],
                axis=mybir.AxisListType.X, op=ALU.add,
            )
            nc.vector.tensor_reduce(
                out=spack[:, 1:2], in_=sqc[:, :],
                axis=mybir.AxisListType.X, op=ALU.add,
            )
            mep = ppb.tile([128, 2], F32, tag="mep", name="mep")
            nc.tensor.matmul(mep[:, :], fm[:, :], spack[:, :], start=True, stop=True)
            me = st.tile([128, 2], F32, tag="me", name="me")
            nc.vector.tensor_copy(me[:, :], mep[:, :])

            # ---- fold stats into weights + bias ---------------------
            msq = st.tile([128, 1], F32, tag="msq", name="msq")
            nc.vector.tensor_tensor(msq[:, :], me[:, 0:1], me[:, 0:1], ALU.mult)
            var = st.tile([128, 1], F32, tag="var", name="var")
            nc.vector.tensor_tensor(var[:, :], me[:, 1:2], msq[:, :], ALU.subtract)
            sd = st.tile([128, 1], F32, tag="sd", name="sd")
            nc.scalar.activation(sd[:, :], var[:, :], ACTF.Sqrt, bias=epst[:, :])
            rs = st.tile([128, 1], F32, tag="rs", name="rs")
            nc.vector.reciprocal(rs[:, :], sd[:, :])
            nc.vector.tensor_scalar_mul(lt[:, :], lt[:, :].bitcast(F32), rs[:, :])
            nmean = st.tile([128, 1], F32, tag="nmean", name="nmean")
            nc.vector.tensor_scalar_mul(nmean[:, :], me[:, 0:1], -1.0)
            pb = ppb.tile([128, 1], F32, tag="pbias", name="pbias")
            nc.tensor.matmul(
                pb[:, :], lt[:, :].bitcast(F32), nmean[:, :], start=True, stop=True
            )
            bp = st.tile([128, 1], F32, tag="bp", name="bp")
            nc.vector.tensor_tensor(bp[:, :], pb[:, :], bt[:, :], ALU.add)

            # ---- grouped conv: block-diag matmul over chunks --------
            mpb = M // n_blk  # batch entries per stage block (per half)
            for b in range(n_blk):
                stg = sp.tile([128, f // n_blk], F32, tag="stage", name=f"stage{b}")
                for c in range(cpb):
                    ch = b * cpb + c
                    ps = pp.tile([128, FC], F32, tag="ps", name=f"ps{ch}")
                    nc.tensor.matmul(
                        ps[:, :],
                        lt[:, :],
                        xt[:, ch * FC : (ch + 1) * FC],
                        start=True,
                        stop=True,
                    )
                    if copy_split and (c % copy_split == copy_split - 1):
                        nc.scalar.activation(
                            stg[:, c * FC : (c + 1) * FC],
                            ps[:, :],
                            ACTF.Identity,
                            bias=bp[:, :],
                        )
                    else:
                        nc.vector.tensor_scalar_add(
                            stg[:, c * FC : (c + 1) * FC], ps[:, :], bp[:, :]
                        )
                sgr = stg.rearrange("p (m f) -> p m f", f=HW)
                for a, eng in ((0, nc.sync), (1, nc.scalar)):
                    nc_slice = sgr[a * 64 : (a + 1) * 64, :, :]
                    eng.dma_start(
                        out=orr[a][:, b * mpb : (b + 1) * mpb, :], in_=nc_slice
                    )

    nc.compile()
    return nc


_NC_CACHE: dict = {}


def _get_nc(n_full: int, n_cores: int):
    key = (n_full, n_cores)
    if key not in _NC_CACHE:
        _NC_CACHE[key] = build_nc(n_full=n_full, n_cores=n_cores)
    return _NC_CACHE[key]


def make_core_inputs(k: int, x, weight, bias, n_cores: int = N_CORES):
    """Host-side shard + derived constants for core k."""
    n_full = x.shape[0]
    cpc = weight.shape[0] // n_cores  # capsules per core
    chl = cpc * D
    ntot = float(n_full * HW)
    lb = np.zeros((128, 128), dtype=np.float32)
    for cl in range(cpc):
        wt = weight[k * cpc + cl].T  # (i, o)
        for a in range(2):
            s = a * 64 + cl * D
            lb[s : s + D, s : s + D] = wt
    fmat = np.zeros((128, 128), dtype=np.float32)
    for p in range(128):
        fmat[p, p] = 1.0 / ntot
        fmat[p, (p + 64) % 128] = 1.0 / ntot
    return {
        "x_shard": np.ascontiguousarray(
            x.reshape(n_full, -1, HW)[:, k * chl : (k + 1) * chl, :]
        ),
        "lhsT_bd": lb,
        "bias_dup": np.tile(
            np.ascontiguousarray(bias[k * chl : (k + 1) * chl]), 2
        ).astype(np.float32),
        "foldmat": fmat,
    }


def make_in_maps(x, weight, bias, n_cores: int = N_CORES):
    return [make_core_inputs(k, x, weight, bias, n_cores) for k in range(n_cores)]


def unshard(outs, n_full: int = N_FULL):
    """Concatenate per-core channel shards back to the full output."""
    full = np.concatenate([o for o in outs], axis=1)  # (n, CD, HW)
    return full.reshape(n_full, CD, H, W)


def kernel(x: np.ndarray, weight: np.ndarray, bias: np.ndarray) -> np.ndarray:
    assert x.shape == (N_FULL, CD, H, W) and x.dtype == np.float32
    nc = _get_nc(N_FULL, N_CORES)
    in_maps = make_in_maps(x, weight, bias)
    res = run_bass_kernel_spmd(nc, in_maps, core_ids=list(range(N_CORES)))
    return unshard([res.results[i]["out"] for i in range(N_CORES)]).astype(
        np.float32, copy=False
    )
